# revision 1
# baseline (speedup 1.0000x reference)
"""2D DCT-II (ortho) over the last two axes of x[8, 32, 512, 512] (f32),
data-parallel across 8 NeuronCores (one batch element per core).

Per core, for each of 32 images X (512x512): Y = D @ X @ D^T.
matmul(out, lhsT, rhs) = lhsT.T @ rhs, so chaining two matmuls with
lhsT = data gives D X D^T with no explicit transposes:
  stage 1: Z = matmul(lhsT=X*, rhs=DT) = (D X*)^T
  stage 2: Y = matmul(lhsT=Z,  rhs=..)
Matmuls run in float32r (tf32-like, 1 cycle/row vs 4 for fp32); inputs
are rounded to f32r by the producing compute ops as the BIR verifier
requires.

Stage 2 is halved with the even/odd DCT split: fold X along its free
dim (Xe/Xo = X[:, i] +/- X[:, 511-i]) before stage 1; then
Y[:, 2k] comes from Ze against A = D[0::2, :256] and Y[:, 2k+1] from
Zo against B = D[1::2, :256], each a 256-contraction. The fold is a
DVE tensor op with a reversed free-dim access pattern.
"""
import numpy as np

import concourse.bass as bass
import concourse.mybir as mybir
import concourse.tile as tile
from concourse.bass_utils import run_bass_kernel_spmd

P = 128
N = 512
H = N // 2          # 256
KO = N // P         # 4
HO = H // P         # 2
NIMG = 32
NCORES = 8
NSPLIT = 1          # partition-range splits per big DMA (splitting below
                    # full 128-partition transfers starves SBUF ports and
                    # drops DMA from ~330 to ~130 GB/s — keep 1)

_MAX_WAITS = 1


def _split_excess_waits(nc):
    """walrus CoreV3 codegen rejects instructions carrying several sem
    waits; hoist excess waits onto preceding same-engine NoOps."""
    for f in nc.m.functions:
        for bb in f.blocks:
            insts = bb.instructions
            i = 0
            while i < len(insts):
                inst = insts[i]
                si = inst.sync_info
                if si is not None and si.on_wait and len(si.on_wait) > _MAX_WAITS:
                    waits = list(si.on_wait)
                    keep = waits[-_MAX_WAITS:]
                    hoist = waits[:-_MAX_WAITS]
                    nops = []
                    for w in hoist:
                        nop = mybir.InstNoOp(
                            name=nc.get_next_instruction_name(), ins=[], outs=[])
                        nop.engine = inst.engine
                        nop.sync_info = mybir.SyncInfo(on_wait=[w], on_update=[])
                        nops.append(nop)
                    si.on_wait = keep
                    for off, nop in enumerate(nops):
                        insts.insert(i + off, nop)
                    i += len(nops)
                i += 1


def _dct_mats(n=N, dtype=np.float32):
    k = np.arange(n)[:, None]
    j = np.arange(n)[None, :]
    D = np.cos(np.pi * (2 * j + 1) * k / (2.0 * n))
    D *= np.sqrt(2.0 / n)
    D[0] *= 1.0 / np.sqrt(2.0)
    D = D.astype(np.float64)
    DT = D.T.astype(dtype)                       # [r, u]
    AT = D[0::2, :H].T.astype(dtype)             # [i, k] even rows
    BT = D[1::2, :H].T.astype(dtype)             # [i, k] odd rows
    return (np.ascontiguousarray(DT), np.ascontiguousarray(AT),
            np.ascontiguousarray(BT))


def _build():
    nc = bass.Bass()
    f32 = mybir.dt.float32
    f32r = mybir.dt.float32r
    x_d = nc.dram_tensor("x", [NIMG, N, N], f32, kind="ExternalInput")
    dt_d = nc.dram_tensor("dt", [N, N], f32, kind="ExternalInput")
    at_d = nc.dram_tensor("at", [H, H], f32, kind="ExternalInput")
    bt_d = nc.dram_tensor("bt", [H, H], f32, kind="ExternalInput")
    y_d = nc.dram_tensor("y", [NIMG, N, N], f32, kind="ExternalOutput")

    PS = P // NSPLIT

    with tile.TileContext(nc) as tc:
        with (
            tc.tile_pool(name="const", bufs=1) as cpool,
            tc.tile_pool(name="xp", bufs=4) as xp,
            tc.tile_pool(name="fp", bufs=3) as fp,
            tc.tile_pool(name="zp", bufs=2) as zp,
            tc.tile_pool(name="yp", bufs=3) as yp,
            tc.tile_pool(name="ps", bufs=4, space="PSUM") as ps1p,
            tc.tile_pool(name="ps2", bufs=4, space="PSUM") as ps2p,
        ):
            dt_f = cpool.tile([P, KO, N], f32, tag="dtf")
            nc.sync.dma_start(dt_f[:], dt_d.rearrange("(ro p) u -> p ro u", p=P))
            dt_mm = cpool.tile([P, KO, N], f32r, tag="dtr")
            nc.vector.tensor_copy(dt_mm[:], dt_f[:])

            ab_f = cpool.tile([P, 2 * HO, H], f32, tag="abf")
            nc.sync.dma_start(
                ab_f[:, 0:HO, :], at_d.rearrange("(io p) k -> p io k", p=P))
            nc.sync.dma_start(
                ab_f[:, HO:2 * HO, :], bt_d.rearrange("(io p) k -> p io k", p=P))
            ab_mm = cpool.tile([P, 2 * HO, H], f32r, tag="abr")
            nc.vector.tensor_copy(ab_mm[:], ab_f[:])
            at_mm = ab_mm[:, 0:HO, :]
            bt_mm = ab_mm[:, HO:2 * HO, :]

            for img in range(NIMG):
                x_sb = xp.tile([P, KO, N], f32)
                x_src = x_d[img].rearrange("(ro p) c -> p ro c", p=P)
                for s in range(NSPLIT):
                    nc.sync.dma_start(
                        x_sb[s * PS:(s + 1) * PS],
                        x_src[s * PS:(s + 1) * PS])

                # free-dim fold -> f32r (the fold op does the rounding)
                xe = fp.tile([P, KO, H], f32r, tag="xe")
                xo = fp.tile([P, KO, H], f32r, tag="xo")
                xrev = x_sb[:, :, N - 1:H - 1:-1]
                nc.vector.tensor_add(xe[:], x_sb[:, :, 0:H], xrev)
                nc.vector.tensor_sub(xo[:], x_sb[:, :, 0:H], xrev)

                # stage 1: Ze/Zo = (D Xe/Xo)^T, [P, 2*HO, N]
                z_sb = zp.tile([P, 2 * HO, N], f32r)
                for part, src in ((0, xe), (1, xo)):
                    for ic in range(HO):
                        pz = ps1p.tile([P, N], f32, tag="ps1")
                        for ro in range(KO):
                            nc.tensor.matmul(
                                pz[:],
                                src[:, ro, ic * P:(ic + 1) * P],
                                dt_mm[:, ro, :],
                                start=(ro == 0),
                                stop=(ro == KO - 1),
                            )
                        if part == 0:
                            nc.vector.tensor_copy(
                                z_sb[:, part * HO + ic, :], pz[:])
                        else:
                            nc.scalar.copy(z_sb[:, part * HO + ic, :], pz[:])
                ze = z_sb[:, 0:HO, :]
                zo = z_sb[:, HO:2 * HO, :]

                # stage 2: psum_y[uo][:, 0:H] even k, [:, H:N] odd k
                y_sb = yp.tile([P, KO, N], f32)
                for uo in range(KO):
                    py = ps2p.tile([P, N], f32, tag="ps2")
                    for ic in range(HO):
                        nc.tensor.matmul(
                            py[:, 0:H],
                            ze[:, ic, uo * P:(uo + 1) * P],
                            at_mm[:, ic, :],
                            start=(ic == 0),
                            stop=(ic == HO - 1),
                        )
                    for ic in range(HO):
                        nc.tensor.matmul(
                            py[:, H:N],
                            zo[:, ic, uo * P:(uo + 1) * P],
                            bt_mm[:, ic, :],
                            start=(ic == 0),
                            stop=(ic == HO - 1),
                        )
                    # interleave: y[p, uo, 2k+t] = py[p, t*H + k]
                    src_ap = py[:].rearrange("p (two k) -> p two k", two=2)
                    dst_ap = y_sb[:, uo, :].rearrange(
                        "p (k two) -> p two k", two=2)
                    if uo != 3:
                        nc.scalar.copy(dst_ap, src_ap)
                    else:
                        nc.vector.tensor_copy(dst_ap, src_ap)
                y_dst = y_d[img].rearrange("(uo p) v -> p uo v", p=P)
                for s in range(NSPLIT):
                    nc.sync.dma_start(
                        y_dst[s * PS:(s + 1) * PS],
                        y_sb[s * PS:(s + 1) * PS])

    _split_excess_waits(nc)
    return nc


_CACHE = {}


def _get_nc():
    if "nc" not in _CACHE:
        _CACHE["nc"] = _build()
    return _CACHE["nc"]


def _in_maps(x):
    dt, at, bt = _dct_mats()
    return [{"x": x[i], "dt": dt, "at": at, "bt": bt} for i in range(NCORES)]


def kernel(x):
    x = np.ascontiguousarray(np.asarray(x, dtype=np.float32))
    assert x.shape == (NCORES, NIMG, N, N), x.shape
    nc = _get_nc()
    res = run_bass_kernel_spmd(nc, _in_maps(x), core_ids=list(range(NCORES)))
    out = np.stack([res.results[i]["y"] for i in range(NCORES)], axis=0)
    return out.astype(np.float32)

